# revision 19
# baseline (speedup 1.0000x reference)
"""Trainium2 Bass kernel for nn_AtteMatchLay (multi-perspective cosine matching).

Math (per flattened row n, perspective p, w2 = weight**2):
    dot[n,p] = sum_d r[n,d]*m[n,d]*w2[p,d]
    n1s[n,p] = sum_d r[n,d]^2 * w2[p,d]
    n2s[n,p] = sum_d m[n,d]^2 * w2[p,d]
    cos[n,p] = (dot * rsqrt(n1s)) * rsqrt(n2s)

Strategy: data-parallel over the flattened N=16*512=8192 rows across 8
cores (1024 rows each). The host packs r|m per (d-block, row-range) piece
into one linear bf16 tensor so every DMA destination is fully contiguous
per partition (fragmented sub-2KB chunks measurably drop HBM read
bandwidth). Nine input pieces stream on the SP HWDGE ring: block 0 split
per row group (fast pipeline prime), blocks 1-4 full, block 5 split into
one half and two quarters so the last-arriving piece is only 128KB and
the serial tail after the final byte is minimal. Per piece: one fused DVE
tensor_tensor computes rm|mm (m broadcast via a stride-0 dim), ACT
squares r, and the PE accumulates dot/n1/n2 into 6 PSUM banks with
region-level stop flags (accumulation is elementwise +=; stop is
bookkeeping only on HW). Seven warmup matmuls on a zeroed tile ramp the
TensorEngine p-state (0.65 -> 1.2 -> 2.4GHz after ~3us of continuous
execution) during the DMA window. Group 1 finishes early and drains its
epilogue + output DMA (on the ACT ring) under group 0's stream; group 0
finishes per quarter, pipelining ACT rsqrt / DVE muls, with one final
output DMA on SP. Outputs are bf16 (cos ~ 0.05; adds ~0.2% rms vs the
2e-2 gate), upcast on host. A dummy Abs_reciprocal_sqrt at t=0 pulls the
ACT table loads off the critical path; exactly 12 DMAs keep the tile
framework from recycling DMA semaphores (a recycle stalls a late input
issue on an early DMA's completion).
"""

import sys

if "/opt/trn_rl_repo" not in sys.path:
    sys.path.insert(0, "/opt/trn_rl_repo")

import numpy as np

# ---- problem constants (hardcoded per contract) ----
BSZ, SL, D, MP = 16, 512, 768, 20
N = BSZ * SL           # 8192 flattened rows
NCORES = 8
NSH = N // NCORES      # 1024 rows per core
P = 128                # SBUF partitions
NB = D // P            # 6 d-blocks
G = 2                  # row groups (fp32 matmul free dim <= 512)
GN = NSH // G          # 512
QN = GN // 2           # 256
# input pieces: (block, row_lo, row_hi); group 1 = rows 512:1024
PIECES = [
    (0, GN, NSH), (0, 0, GN),
    (1, 0, NSH), (2, 0, NSH), (3, 0, NSH), (4, 0, NSH),
    (5, GN, NSH), (5, QN, GN), (5, 0, QN),
]
NWARM = 7              # PE p-state warmup matmuls

_CACHE = {}


def _build():
    import concourse.tile as tile
    from concourse import bacc, mybir

    f32 = mybir.dt.float32
    bf16 = mybir.dt.bfloat16
    nc = bacc.Bacc(None, target_bir_lowering=False)

    # x rows i*P..(i+1)*P hold piece i: per partition [r rows | m rows].
    x = nc.dram_tensor("x", [len(PIECES) * P, 2 * NSH], bf16, kind="ExternalInput")
    # w2p[p, b*MP + q] = w2[q, b*128+p]; padded to 512 cols (128KB) — the
    # oversized first transfer doubles as a DMA-path warmup.
    w2p = nc.dram_tensor("w2p", [P, 512], bf16, kind="ExternalInput")
    out = nc.dram_tensor("out", [MP, NSH], bf16, kind="ExternalOutput")

    SQ = mybir.ActivationFunctionType.Square
    ARSQRT = mybir.ActivationFunctionType.Abs_reciprocal_sqrt
    MUL = mybir.AluOpType.mult

    with tile.TileContext(nc) as tc:
        with (
            tc.tile_pool(name="const", bufs=1) as const,
            tc.tile_pool(name="inp", bufs=1) as inp,
            tc.tile_pool(name="prod", bufs=3) as prod,
            tc.tile_pool(name="epi", bufs=2) as epi,
            tc.tile_pool(name="psum", bufs=1, space="PSUM") as psum,
        ):
            # w2 on the ACT HWDGE ring so SP's first input DMA issues at t=0.
            w2_sb = const.tile([P, 512], bf16, tag="w2")
            nc.scalar.dma_start(out=w2_sb[:], in_=w2p[:, :])

            # Dummy activation: pulls the ACT table loads to t~0.
            dum_i = const.tile([1, 8], f32, tag="dum_i")
            dum_o = const.tile([1, 8], f32, tag="dum_o")
            nc.gpsimd.memset(dum_i[:], 1.0)
            nc.scalar.activation(dum_o[:], dum_i[:], ARSQRT)

            # PE warmup on a zeroed tile.
            zt = const.tile([P, MP + GN], bf16, tag="zt")
            nc.gpsimd.memset(zt[:], 0.0)
            warm_ps = psum.tile([MP, GN], f32, name="warm", tag="warm")
            for _ in range(NWARM):
                nc.tensor.matmul(
                    warm_ps[:], zt[:, 0:MP], zt[:, MP : MP + GN],
                    start=True, stop=True,
                )

            # Input stream: all pieces on the SP ring, contiguous dst.
            xsb = inp.tile([P, len(PIECES), 2 * NSH], bf16, tag="x")
            for i, (b, lo, hi) in enumerate(PIECES):
                w = hi - lo
                nc.sync.dma_start(
                    out=xsb[:, i, 0 : 2 * w],
                    in_=x[i * P : (i + 1) * P, 0 : 2 * w],
                )

            dot_ps = [psum.tile([MP, GN], f32, name=f"dot{g}", tag=f"dot{g}") for g in range(G)]
            n1_ps = [psum.tile([MP, GN], f32, name=f"n1{g}", tag=f"n1{g}") for g in range(G)]
            n2_ps = [psum.tile([MP, GN], f32, name=f"n2{g}", tag=f"n2{g}") for g in range(G)]
            isq1 = [epi.tile([MP, GN], f32, name=f"i1{g}", tag=f"i1{g}") for g in range(G)]
            isq2 = [epi.tile([MP, GN], f32, name=f"i2{g}", tag=f"i2{g}") for g in range(G)]
            d2 = [epi.tile([MP, GN], f32, name=f"d2{g}", tag=f"d2{g}") for g in range(G)]
            cos = [epi.tile([MP, GN], bf16, name=f"cos{g}", tag=f"cos{g}") for g in range(G)]

            def epi_chain(g, qlo, qhi):
                # cos[qlo:qhi] = (dot * rsqrt(n1s)) * rsqrt(n2s); isq1/d2
                # hide under the n2 matmuls (n1 and dot stop earlier).
                qs = slice(qlo, qhi)
                nc.scalar.activation(isq1[g][:, qs], n1_ps[g][:, qs], ARSQRT)
                nc.vector.tensor_tensor(d2[g][:, qs], dot_ps[g][:, qs], isq1[g][:, qs], MUL)
                nc.scalar.activation(isq2[g][:, qs], n2_ps[g][:, qs], ARSQRT)
                nc.vector.tensor_tensor(cos[g][:, qs], d2[g][:, qs], isq2[g][:, qs], MUL)

            for i, (b, lo, hi) in enumerate(PIECES):
                w = hi - lo
                w2b = w2_sb[:, b * MP : (b + 1) * MP]
                # Within piece i: r rows at cols [0:w], m rows at [w:2w].
                xt = xsb[:, i, 0 : 2 * w].rearrange("p (t n) -> p t n", t=2)
                # Per-group sub-ranges of this piece.
                subs = []
                for g in range(G):
                    s_lo = max(lo, g * GN)
                    s_hi = min(hi, (g + 1) * GN)
                    if s_lo < s_hi:
                        subs.append((g, s_lo, s_hi))
                # g1 first within full pieces so g1 stops before g0 at b5...
                # (irrelevant for full pieces; kept in ascending-g order)
                for g, s_lo, s_hi in subs:
                    c = s_lo - lo
                    sw = s_hi - s_lo
                    pm = prod.tile([P, 2, sw], bf16, name=f"pm_{i}_{g}", tag=f"pm{sw}")
                    in0 = xt[:, :, c : c + sw]
                    in1 = xt[:, 1, c : c + sw].unsqueeze(1).broadcast_to([P, 2, sw])
                    nc.vector.tensor_tensor(pm[:], in0, in1, MUL)
                    rr = prod.tile([P, sw], bf16, name=f"rr_{i}_{g}", tag=f"rr{sw}")
                    nc.scalar.activation(rr[:], xt[:, 0, c : c + sw], SQ)

                    st, sp = b == 0, b == NB - 1
                    qs = slice(s_lo - g * GN, s_hi - g * GN)
                    # dot first (gated only on DVE's pm), n1 next, n2 last.
                    nc.tensor.matmul(dot_ps[g][:, qs], w2b, pm[:, 0, :],
                                     start=st, stop=sp, skip_group_check=True)
                    nc.tensor.matmul(n1_ps[g][:, qs], w2b, rr[:],
                                     start=st, stop=sp, skip_group_check=True)
                    nc.tensor.matmul(n2_ps[g][:, qs], w2b, pm[:, 1, :],
                                     start=st, stop=sp, skip_group_check=True)

                if b == NB - 1:
                    if (lo, hi) == (GN, NSH):
                        # group 1 complete: epilogue + output on the ACT
                        # ring, hidden under group 0's remaining stream.
                        epi_chain(1, 0, GN)
                        nc.scalar.dma_start(out=out[:, GN:NSH], in_=cos[1][:])
                    else:
                        # group 0 quarters as their stops land
                        epi_chain(0, lo, hi)

            # single output DMA for group 0 (both quarters)
            nc.sync.dma_start(out=out[:, 0:GN], in_=cos[0][:])

    nc.finalize()
    return nc


def get_nc():
    if "nc" not in _CACHE:
        _CACHE["nc"] = _build()
    return _CACHE["nc"]


def make_in_maps(repres, max_att, weight):
    import ml_dtypes

    bf16 = ml_dtypes.bfloat16
    r = np.ascontiguousarray(repres, dtype=np.float32).reshape(N, D)
    m = np.ascontiguousarray(max_att, dtype=np.float32).reshape(N, D)
    # w2p[p, b*MP+q] = w2[q, b*128+p], padded to 512 cols
    w2 = (weight.astype(np.float32) ** 2).T  # [D, MP]
    w2p = np.zeros((P, 512), dtype=bf16)
    w2p[:, : NB * MP] = (
        w2.reshape(NB, P, MP).transpose(1, 0, 2).reshape(P, NB * MP).astype(bf16)
    )
    w2p = np.ascontiguousarray(w2p)
    in_maps = []
    for c in range(NCORES):
        rows = slice(c * NSH, (c + 1) * NSH)
        rT = r[rows].T.reshape(NB, P, NSH).astype(bf16)  # [b][p][n]
        mT = m[rows].T.reshape(NB, P, NSH).astype(bf16)
        xc = np.zeros((len(PIECES), P, 2 * NSH), dtype=bf16)
        for i, (b, lo, hi) in enumerate(PIECES):
            w = hi - lo
            xc[i, :, 0:w] = rT[b, :, lo:hi]
            xc[i, :, w : 2 * w] = mT[b, :, lo:hi]
        in_maps.append(
            {
                "x": np.ascontiguousarray(xc.reshape(len(PIECES) * P, 2 * NSH)),
                "w2p": w2p,
            }
        )
    return in_maps


def gather(results):
    # results: list of dicts with "out" [MP, NSH] bf16 per core -> [BSZ, SL, MP] f32
    cols = np.concatenate(
        [results[c]["out"].astype(np.float32) for c in range(NCORES)], axis=1
    )
    return np.ascontiguousarray(cols.T).reshape(BSZ, SL, MP)


def kernel(repres, max_att, weight, **kw):
    from concourse.bass_utils import run_bass_kernel_spmd

    nc = get_nc()
    in_maps = make_in_maps(repres, max_att, weight)
    res = run_bass_kernel_spmd(nc, in_maps, list(range(NCORES)))
    return gather(res.results)


# revision 20
# speedup vs baseline: 1.0526x; 1.0526x over previous
"""Trainium2 Bass kernel for nn_AtteMatchLay (multi-perspective cosine matching).

Math (per flattened row n, perspective p, w2 = weight**2):
    dot[n,p] = sum_d r[n,d]*m[n,d]*w2[p,d]
    n1s[n,p] = sum_d r[n,d]^2 * w2[p,d]
    n2s[n,p] = sum_d m[n,d]^2 * w2[p,d]
    cos[n,p] = (dot * rsqrt(n1s)) * rsqrt(n2s)

Strategy: data-parallel over the flattened N=16*512=8192 rows across 8
cores (1024 rows each). The host packs r|m per (d-block, row-range) piece
into one linear bf16 tensor so every DMA destination is fully contiguous
per partition (fragmented sub-2KB chunks measurably drop HBM read
bandwidth). Nine input pieces stream on the SP HWDGE ring: block 0 split
per row group (fast pipeline prime), blocks 1-4 full, block 5 split into
one half and two quarters so the last-arriving piece is only 128KB and
the serial tail after the final byte is minimal. Per piece: one fused DVE
tensor_tensor computes rm|mm (m broadcast via a stride-0 dim), ACT
squares r, and the PE accumulates dot/n1/n2 into 6 PSUM banks with
region-level stop flags (accumulation is elementwise +=; stop is
bookkeeping only on HW). Seven warmup matmuls on a zeroed tile ramp the
TensorEngine p-state (0.65 -> 1.2 -> 2.4GHz after ~3us of continuous
execution) during the DMA window. Group 1 finishes early and drains its
epilogue + output DMA (on the ACT ring) under group 0's stream; group 0
finishes per quarter, pipelining ACT rsqrt / DVE muls, with one final
output DMA on SP. Outputs are bf16 (cos ~ 0.05; adds ~0.2% rms vs the
2e-2 gate), upcast on host. A dummy Abs_reciprocal_sqrt at t=0 pulls the
ACT table loads off the critical path; exactly 12 DMAs keep the tile
framework from recycling DMA semaphores (a recycle stalls a late input
issue on an early DMA's completion).
"""

import sys

if "/opt/trn_rl_repo" not in sys.path:
    sys.path.insert(0, "/opt/trn_rl_repo")

import numpy as np

# ---- problem constants (hardcoded per contract) ----
BSZ, SL, D, MP = 16, 512, 768, 20
N = BSZ * SL           # 8192 flattened rows
NCORES = 8
NSH = N // NCORES      # 1024 rows per core
P = 128                # SBUF partitions
NB = D // P            # 6 d-blocks
G = 2                  # row groups (fp32 matmul free dim <= 512)
GN = NSH // G          # 512
QN = GN // 2           # 256
# input pieces: (block, row_lo, row_hi); group 1 = rows 512:1024
PIECES = [
    (0, GN, NSH), (0, 0, GN),
    (1, 0, NSH), (2, 0, NSH), (3, 0, NSH), (4, 0, NSH),
    (5, GN, NSH), (5, 0, GN),
]
NWARM = 7              # PE p-state warmup matmuls

_CACHE = {}


def _build():
    import concourse.tile as tile
    from concourse import bacc, mybir

    f32 = mybir.dt.float32
    bf16 = mybir.dt.bfloat16
    nc = bacc.Bacc(None, target_bir_lowering=False)

    # x rows i*P..(i+1)*P hold piece i: per partition [r rows | m rows].
    x = nc.dram_tensor("x", [len(PIECES) * P, 2 * NSH], bf16, kind="ExternalInput")
    # w2p[p, b*MP + q] = w2[q, b*128+p]; padded to 512 cols (128KB) — the
    # oversized first transfer doubles as a DMA-path warmup.
    w2p = nc.dram_tensor("w2p", [P, 512], bf16, kind="ExternalInput")
    out = nc.dram_tensor("out", [MP, NSH], bf16, kind="ExternalOutput")

    SQ = mybir.ActivationFunctionType.Square
    ARSQRT = mybir.ActivationFunctionType.Abs_reciprocal_sqrt
    MUL = mybir.AluOpType.mult

    with tile.TileContext(nc) as tc:
        with (
            tc.tile_pool(name="const", bufs=1) as const,
            tc.tile_pool(name="inp", bufs=1) as inp,
            tc.tile_pool(name="prod", bufs=3) as prod,
            tc.tile_pool(name="epi", bufs=2) as epi,
            tc.tile_pool(name="psum", bufs=1, space="PSUM") as psum,
        ):
            # w2 on the ACT HWDGE ring so SP's first input DMA issues at t=0.
            w2_sb = const.tile([P, 512], bf16, tag="w2")
            nc.scalar.dma_start(out=w2_sb[:], in_=w2p[:, :])

            # Dummy activation: pulls the ACT table loads to t~0.
            dum_i = const.tile([1, 8], f32, tag="dum_i")
            dum_o = const.tile([1, 8], f32, tag="dum_o")
            nc.gpsimd.memset(dum_i[:], 1.0)
            nc.scalar.activation(dum_o[:], dum_i[:], ARSQRT)

            # PE warmup on a zeroed tile.
            zt = const.tile([P, MP + GN], bf16, tag="zt")
            nc.gpsimd.memset(zt[:], 0.0)
            warm_ps = psum.tile([MP, GN], f32, name="warm", tag="warm")
            for _ in range(NWARM):
                nc.tensor.matmul(
                    warm_ps[:], zt[:, 0:MP], zt[:, MP : MP + GN],
                    start=True, stop=True,
                )

            # Input stream: all pieces on the SP ring, contiguous dst.
            xsb = inp.tile([P, len(PIECES), 2 * NSH], bf16, tag="x")
            for i, (b, lo, hi) in enumerate(PIECES):
                w = hi - lo
                nc.sync.dma_start(
                    out=xsb[:, i, 0 : 2 * w],
                    in_=x[i * P : (i + 1) * P, 0 : 2 * w],
                )

            dot_ps = [psum.tile([MP, GN], f32, name=f"dot{g}", tag=f"dot{g}") for g in range(G)]
            n1_ps = [psum.tile([MP, GN], f32, name=f"n1{g}", tag=f"n1{g}") for g in range(G)]
            n2_ps = [psum.tile([MP, GN], f32, name=f"n2{g}", tag=f"n2{g}") for g in range(G)]
            isq1 = [epi.tile([MP, GN], f32, name=f"i1{g}", tag=f"i1{g}") for g in range(G)]
            isq2 = [epi.tile([MP, GN], f32, name=f"i2{g}", tag=f"i2{g}") for g in range(G)]
            d2 = [epi.tile([MP, GN], f32, name=f"d2{g}", tag=f"d2{g}") for g in range(G)]
            cos = [epi.tile([MP, GN], bf16, name=f"cos{g}", tag=f"cos{g}") for g in range(G)]

            def epi_chain(g, qlo, qhi):
                # cos[qlo:qhi] = (dot * rsqrt(n1s)) * rsqrt(n2s); isq1/d2
                # hide under the n2 matmuls (n1 and dot stop earlier).
                qs = slice(qlo, qhi)
                nc.scalar.activation(isq1[g][:, qs], n1_ps[g][:, qs], ARSQRT)
                nc.vector.tensor_tensor(d2[g][:, qs], dot_ps[g][:, qs], isq1[g][:, qs], MUL)
                nc.scalar.activation(isq2[g][:, qs], n2_ps[g][:, qs], ARSQRT)
                nc.vector.tensor_tensor(cos[g][:, qs], d2[g][:, qs], isq2[g][:, qs], MUL)

            for i, (b, lo, hi) in enumerate(PIECES):
                w = hi - lo
                w2b = w2_sb[:, b * MP : (b + 1) * MP]
                # Within piece i: r rows at cols [0:w], m rows at [w:2w].
                xt = xsb[:, i, 0 : 2 * w].rearrange("p (t n) -> p t n", t=2)
                # Per-group sub-ranges of this piece.
                subs = []
                for g in range(G):
                    s_lo = max(lo, g * GN)
                    s_hi = min(hi, (g + 1) * GN)
                    if s_lo < s_hi:
                        subs.append((g, s_lo, s_hi))
                # g1 first within full pieces so g1 stops before g0 at b5...
                # (irrelevant for full pieces; kept in ascending-g order)
                for g, s_lo, s_hi in subs:
                    c = s_lo - lo
                    sw = s_hi - s_lo
                    pm = prod.tile([P, 2, sw], bf16, name=f"pm_{i}_{g}", tag=f"pm{sw}")
                    in0 = xt[:, :, c : c + sw]
                    in1 = xt[:, 1, c : c + sw].unsqueeze(1).broadcast_to([P, 2, sw])
                    nc.vector.tensor_tensor(pm[:], in0, in1, MUL)
                    rr = prod.tile([P, sw], bf16, name=f"rr_{i}_{g}", tag=f"rr{sw}")
                    nc.scalar.activation(rr[:], xt[:, 0, c : c + sw], SQ)

                    st, sp = b == 0, b == NB - 1
                    qs = slice(s_lo - g * GN, s_hi - g * GN)
                    # dot first (gated only on DVE's pm), n1 next, n2 last.
                    nc.tensor.matmul(dot_ps[g][:, qs], w2b, pm[:, 0, :],
                                     start=st, stop=sp, skip_group_check=True)
                    nc.tensor.matmul(n1_ps[g][:, qs], w2b, rr[:],
                                     start=st, stop=sp, skip_group_check=True)
                    nc.tensor.matmul(n2_ps[g][:, qs], w2b, pm[:, 1, :],
                                     start=st, stop=sp, skip_group_check=True)

                if b == NB - 1:
                    if (lo, hi) == (GN, NSH):
                        # group 1 complete: epilogue + output hidden under
                        # group 0's remaining stream.
                        epi_chain(1, 0, GN)
                        nc.sync.dma_start(out=out[:, GN:NSH], in_=cos[1][:])
                    else:
                        # group 0: two quarter chains pipeline ACT/DVE
                        epi_chain(0, 0, QN)
                        epi_chain(0, QN, GN)

            # single output DMA for group 0 (both quarters)
            nc.sync.dma_start(out=out[:, 0:GN], in_=cos[0][:])

    nc.finalize()
    return nc


def get_nc():
    if "nc" not in _CACHE:
        _CACHE["nc"] = _build()
    return _CACHE["nc"]


def make_in_maps(repres, max_att, weight):
    import ml_dtypes

    bf16 = ml_dtypes.bfloat16
    r = np.ascontiguousarray(repres, dtype=np.float32).reshape(N, D)
    m = np.ascontiguousarray(max_att, dtype=np.float32).reshape(N, D)
    # w2p[p, b*MP+q] = w2[q, b*128+p], padded to 512 cols
    w2 = (weight.astype(np.float32) ** 2).T  # [D, MP]
    w2p = np.zeros((P, 512), dtype=bf16)
    w2p[:, : NB * MP] = (
        w2.reshape(NB, P, MP).transpose(1, 0, 2).reshape(P, NB * MP).astype(bf16)
    )
    w2p = np.ascontiguousarray(w2p)
    in_maps = []
    for c in range(NCORES):
        rows = slice(c * NSH, (c + 1) * NSH)
        rT = r[rows].T.reshape(NB, P, NSH).astype(bf16)  # [b][p][n]
        mT = m[rows].T.reshape(NB, P, NSH).astype(bf16)
        xc = np.zeros((len(PIECES), P, 2 * NSH), dtype=bf16)
        for i, (b, lo, hi) in enumerate(PIECES):
            w = hi - lo
            xc[i, :, 0:w] = rT[b, :, lo:hi]
            xc[i, :, w : 2 * w] = mT[b, :, lo:hi]
        in_maps.append(
            {
                "x": np.ascontiguousarray(xc.reshape(len(PIECES) * P, 2 * NSH)),
                "w2p": w2p,
            }
        )
    return in_maps


def gather(results):
    # results: list of dicts with "out" [MP, NSH] bf16 per core -> [BSZ, SL, MP] f32
    cols = np.concatenate(
        [results[c]["out"].astype(np.float32) for c in range(NCORES)], axis=1
    )
    return np.ascontiguousarray(cols.T).reshape(BSZ, SL, MP)


def kernel(repres, max_att, weight, **kw):
    from concourse.bass_utils import run_bass_kernel_spmd

    nc = get_nc()
    in_maps = make_in_maps(repres, max_att, weight)
    res = run_bass_kernel_spmd(nc, in_maps, list(range(NCORES)))
    return gather(res.results)


# revision 21
# speedup vs baseline: 1.0569x; 1.0041x over previous
"""Trainium2 Bass kernel for nn_AtteMatchLay (multi-perspective cosine matching).

Math (per flattened row n, perspective p, w2 = weight**2):
    dot[n,p] = sum_d r[n,d]*m[n,d]*w2[p,d]
    n1s[n,p] = sum_d r[n,d]^2 * w2[p,d]
    n2s[n,p] = sum_d m[n,d]^2 * w2[p,d]
    cos[n,p] = (dot * rsqrt(n1s)) * rsqrt(n2s)

Strategy: data-parallel over the flattened N=16*512=8192 rows across 8
cores (1024 rows each). The host packs r|m per (d-block, row-range) piece
into one linear bf16 tensor so every DMA destination is fully contiguous
per partition (fragmented sub-2KB chunks measurably drop HBM read
bandwidth). Nine input pieces stream on the SP HWDGE ring: block 0 split
per row group (fast pipeline prime), blocks 1-4 full, block 5 split into
one half and two quarters so the last-arriving piece is only 128KB and
the serial tail after the final byte is minimal. Per piece: one fused DVE
tensor_tensor computes rm|mm (m broadcast via a stride-0 dim), ACT
squares r, and the PE accumulates dot/n1/n2 into 6 PSUM banks with
region-level stop flags (accumulation is elementwise +=; stop is
bookkeeping only on HW). Seven warmup matmuls on a zeroed tile ramp the
TensorEngine p-state (0.65 -> 1.2 -> 2.4GHz after ~3us of continuous
execution) during the DMA window. Group 1 finishes early and drains its
epilogue + output DMA (on the ACT ring) under group 0's stream; group 0
finishes per quarter, pipelining ACT rsqrt / DVE muls, with one final
output DMA on SP. Outputs are bf16 (cos ~ 0.05; adds ~0.2% rms vs the
2e-2 gate), upcast on host. A dummy Abs_reciprocal_sqrt at t=0 pulls the
ACT table loads off the critical path; exactly 12 DMAs keep the tile
framework from recycling DMA semaphores (a recycle stalls a late input
issue on an early DMA's completion).
"""

import sys

if "/opt/trn_rl_repo" not in sys.path:
    sys.path.insert(0, "/opt/trn_rl_repo")

import numpy as np

# ---- problem constants (hardcoded per contract) ----
BSZ, SL, D, MP = 16, 512, 768, 20
N = BSZ * SL           # 8192 flattened rows
NCORES = 8
NSH = N // NCORES      # 1024 rows per core
P = 128                # SBUF partitions
NB = D // P            # 6 d-blocks
G = 2                  # row groups (fp32 matmul free dim <= 512)
GN = NSH // G          # 512
QN = GN // 2           # 256
# input pieces: (block, row_lo, row_hi); group 1 = rows 512:1024
PIECES = [
    (0, GN, NSH), (0, 0, GN),
    (1, 0, NSH), (2, 0, NSH), (3, 0, NSH), (4, 0, NSH),
    (5, GN, NSH), (5, 0, GN),
]
NWARM = 7              # PE p-state warmup matmuls

_CACHE = {}


def _build():
    import concourse.tile as tile
    from concourse import bacc, mybir

    f32 = mybir.dt.float32
    bf16 = mybir.dt.bfloat16
    nc = bacc.Bacc(None, target_bir_lowering=False)

    # x rows i*P..(i+1)*P hold piece i: per partition [r rows | m rows].
    x = nc.dram_tensor("x", [len(PIECES) * P, 2 * NSH], bf16, kind="ExternalInput")
    # w2p[p, b*MP + q] = w2[q, b*128+p]
    w2p = nc.dram_tensor("w2p", [P, NB * MP], bf16, kind="ExternalInput")
    out = nc.dram_tensor("out", [MP, NSH], bf16, kind="ExternalOutput")

    SQ = mybir.ActivationFunctionType.Square
    ARSQRT = mybir.ActivationFunctionType.Abs_reciprocal_sqrt
    MUL = mybir.AluOpType.mult

    with tile.TileContext(nc) as tc:
        with (
            tc.tile_pool(name="const", bufs=1) as const,
            tc.tile_pool(name="inp", bufs=1) as inp,
            tc.tile_pool(name="prod", bufs=3) as prod,
            tc.tile_pool(name="epi", bufs=2) as epi,
            tc.tile_pool(name="psum", bufs=1, space="PSUM") as psum,
        ):
            # w2 on the ACT HWDGE ring so SP's first input DMA issues at t=0.
            w2_sb = const.tile([P, NB * MP], bf16, tag="w2")
            nc.scalar.dma_start(out=w2_sb[:], in_=w2p[:, :])

            # Dummy activation: pulls the ACT table loads to t~0.
            dum_i = const.tile([1, 8], f32, tag="dum_i")
            dum_o = const.tile([1, 8], f32, tag="dum_o")
            nc.gpsimd.memset(dum_i[:], 1.0)
            nc.scalar.activation(dum_o[:], dum_i[:], ARSQRT)

            # PE warmup on a zeroed tile.
            zt = const.tile([P, MP + GN], bf16, tag="zt")
            nc.gpsimd.memset(zt[:], 0.0)
            warm_ps = psum.tile([MP, GN], f32, name="warm", tag="warm")
            for _ in range(NWARM):
                nc.tensor.matmul(
                    warm_ps[:], zt[:, 0:MP], zt[:, MP : MP + GN],
                    start=True, stop=True,
                )

            # Input stream: all pieces on the SP ring, contiguous dst.
            xsb = inp.tile([P, len(PIECES), 2 * NSH], bf16, tag="x")
            for i, (b, lo, hi) in enumerate(PIECES):
                w = hi - lo
                nc.sync.dma_start(
                    out=xsb[:, i, 0 : 2 * w],
                    in_=x[i * P : (i + 1) * P, 0 : 2 * w],
                )

            dot_ps = [psum.tile([MP, GN], f32, name=f"dot{g}", tag=f"dot{g}") for g in range(G)]
            n1_ps = [psum.tile([MP, GN], f32, name=f"n1{g}", tag=f"n1{g}") for g in range(G)]
            n2_ps = [psum.tile([MP, GN], f32, name=f"n2{g}", tag=f"n2{g}") for g in range(G)]
            isq1 = [epi.tile([MP, GN], f32, name=f"i1{g}", tag=f"i1{g}") for g in range(G)]
            isq2 = [epi.tile([MP, GN], f32, name=f"i2{g}", tag=f"i2{g}") for g in range(G)]
            d2 = [epi.tile([MP, GN], f32, name=f"d2{g}", tag=f"d2{g}") for g in range(G)]
            cos = [epi.tile([MP, GN], bf16, name=f"cos{g}", tag=f"cos{g}") for g in range(G)]

            def epi_chain(g, qlo, qhi):
                # cos[qlo:qhi] = (dot * rsqrt(n1s)) * rsqrt(n2s); isq1/d2
                # hide under the n2 matmuls (n1 and dot stop earlier).
                qs = slice(qlo, qhi)
                nc.scalar.activation(isq1[g][:, qs], n1_ps[g][:, qs], ARSQRT)
                nc.vector.tensor_tensor(d2[g][:, qs], dot_ps[g][:, qs], isq1[g][:, qs], MUL)
                nc.scalar.activation(isq2[g][:, qs], n2_ps[g][:, qs], ARSQRT)
                nc.vector.tensor_tensor(cos[g][:, qs], d2[g][:, qs], isq2[g][:, qs], MUL)

            for i, (b, lo, hi) in enumerate(PIECES):
                w = hi - lo
                w2b = w2_sb[:, b * MP : (b + 1) * MP]
                # Within piece i: r rows at cols [0:w], m rows at [w:2w].
                xt = xsb[:, i, 0 : 2 * w].rearrange("p (t n) -> p t n", t=2)
                # Per-group sub-ranges of this piece.
                subs = []
                for g in range(G):
                    s_lo = max(lo, g * GN)
                    s_hi = min(hi, (g + 1) * GN)
                    if s_lo < s_hi:
                        subs.append((g, s_lo, s_hi))
                # g1 first within full pieces so g1 stops before g0 at b5...
                # (irrelevant for full pieces; kept in ascending-g order)
                for g, s_lo, s_hi in subs:
                    c = s_lo - lo
                    sw = s_hi - s_lo
                    pm = prod.tile([P, 2, sw], bf16, name=f"pm_{i}_{g}", tag=f"pm{sw}")
                    in0 = xt[:, :, c : c + sw]
                    in1 = xt[:, 1, c : c + sw].unsqueeze(1).broadcast_to([P, 2, sw])
                    nc.vector.tensor_tensor(pm[:], in0, in1, MUL)
                    rr = prod.tile([P, sw], bf16, name=f"rr_{i}_{g}", tag=f"rr{sw}")
                    nc.scalar.activation(rr[:], xt[:, 0, c : c + sw], SQ)

                    st, sp = b == 0, b == NB - 1
                    qs = slice(s_lo - g * GN, s_hi - g * GN)
                    # dot first (gated only on DVE's pm), n1 next, n2 last.
                    nc.tensor.matmul(dot_ps[g][:, qs], w2b, pm[:, 0, :],
                                     start=st, stop=sp, skip_group_check=True)
                    nc.tensor.matmul(n1_ps[g][:, qs], w2b, rr[:],
                                     start=st, stop=sp, skip_group_check=True)
                    nc.tensor.matmul(n2_ps[g][:, qs], w2b, pm[:, 1, :],
                                     start=st, stop=sp, skip_group_check=True)

                if b == NB - 1:
                    if (lo, hi) == (GN, NSH):
                        # group 1 complete: epilogue + output hidden under
                        # group 0's remaining stream.
                        epi_chain(1, 0, GN)
                        nc.sync.dma_start(out=out[:, GN:NSH], in_=cos[1][:])
                    else:
                        # group 0: two quarter chains pipeline ACT/DVE
                        epi_chain(0, 0, QN)
                        epi_chain(0, QN, GN)

            # single output DMA for group 0 (both quarters)
            nc.sync.dma_start(out=out[:, 0:GN], in_=cos[0][:])

    nc.finalize()
    return nc


def get_nc():
    if "nc" not in _CACHE:
        _CACHE["nc"] = _build()
    return _CACHE["nc"]


def make_in_maps(repres, max_att, weight):
    import ml_dtypes

    bf16 = ml_dtypes.bfloat16
    r = np.ascontiguousarray(repres, dtype=np.float32).reshape(N, D)
    m = np.ascontiguousarray(max_att, dtype=np.float32).reshape(N, D)
    # w2p[p, b*MP+q] = w2[q, b*128+p], padded to 512 cols
    w2 = (weight.astype(np.float32) ** 2).T  # [D, MP]
    w2p = np.ascontiguousarray(
        w2.reshape(NB, P, MP).transpose(1, 0, 2).reshape(P, NB * MP).astype(bf16)
    )
    in_maps = []
    for c in range(NCORES):
        rows = slice(c * NSH, (c + 1) * NSH)
        rT = r[rows].T.reshape(NB, P, NSH).astype(bf16)  # [b][p][n]
        mT = m[rows].T.reshape(NB, P, NSH).astype(bf16)
        xc = np.zeros((len(PIECES), P, 2 * NSH), dtype=bf16)
        for i, (b, lo, hi) in enumerate(PIECES):
            w = hi - lo
            xc[i, :, 0:w] = rT[b, :, lo:hi]
            xc[i, :, w : 2 * w] = mT[b, :, lo:hi]
        in_maps.append(
            {
                "x": np.ascontiguousarray(xc.reshape(len(PIECES) * P, 2 * NSH)),
                "w2p": w2p,
            }
        )
    return in_maps


def gather(results):
    # results: list of dicts with "out" [MP, NSH] bf16 per core -> [BSZ, SL, MP] f32
    cols = np.concatenate(
        [results[c]["out"].astype(np.float32) for c in range(NCORES)], axis=1
    )
    return np.ascontiguousarray(cols.T).reshape(BSZ, SL, MP)


def kernel(repres, max_att, weight, **kw):
    from concourse.bass_utils import run_bass_kernel_spmd

    nc = get_nc()
    in_maps = make_in_maps(repres, max_att, weight)
    res = run_bass_kernel_spmd(nc, in_maps, list(range(NCORES)))
    return gather(res.results)
